# revision 1
# baseline (speedup 1.0000x reference)
"""Neural CDE (Tsit5 scan over cubic-interp control) on 8 Trainium2 cores.

Strategy: pure data parallelism over batch (64 -> 8 per core). Everything
resident in SBUF; per stage: 3 matmul layers (bf16 weights, f32 psum),
softplus via ACT(abs,exp) + fused custom DVE tail, tanh via ACT, control
einsum via broadcast-DMA'd dxdt row + gpsimd multiply + DVE reduce, and the
Runge-Kutta linear combinations via a weighted multiply+reduce against
resident coefficient tiles. The control derivative dxdt(t) at all 756 stage
times is precomputed on host (all stage times are compile-time constants)
and shipped compactly as one [756, 256] f32 table per core.
"""

import numpy as np
import ml_dtypes
from contextlib import ExitStack

bf16 = ml_dtypes.bfloat16

# ---- problem constants (hardcoded per spec) ----
B, T, IN, H, WID, OUT = 64, 64, 32, 128, 128, 1
SUBSTEPS = 2
N_STEPS = (T - 1) * SUBSTEPS  # 126
N_CORES = 8
BL = B // N_CORES  # 8 batch per core
NST = N_STEPS * 6  # 756 stage evals

# Tsit5 tableau
C2, C3, C4, C5 = 0.161, 0.327, 0.9, 0.9800255409045097
A_ROWS = [
    [0.161],
    [-0.008480655492356989, 0.335480655492357],
    [2.8971530571054935, -6.359448489975075, 4.3622954328695815],
    [5.325864828439257, -11.748883564062828, 7.4955393428898365, -0.09249506636175525],
    [5.86145544294642, -12.92096931784711, 8.159367898576159, -0.071584973281401,
     -0.028269050394068383],
]
B_ROW = [0.09646076681806523, 0.01, 0.4798896504144996, 1.379008574103742,
         -3.290069515436081, 2.324710524099774]
C_OFFS = [0.0, C2, C3, C4, C5, 1.0]

# log1p(u) ~= u * (1 + u*(Q1 + u*Q2)) on (0, 1]  (max abs err ~1.3e-3)
Q1 = -0.44593992199872445
Q2 = 0.14039984369167596
# e^{-a} ~= (1 + a*(P4_1 + a*(P4_2 + a*(P4_3 + a*P4_4))))^4 on [0, 8]
# (abs err ~6e-5); a clamped to 8 (tail contribution < 3.4e-4)
P4 = [-0.24985221943552863, 0.030860666100303438,
      -0.0023213489733239527, 8.616820461546782e-05]
A_CLAMP = 8.0

_CACHE = {}


def _register_custom_ops():
    from concourse.dve_spec import (Spec, Src0, Src1, C0, C1, C2 as C2L, One,
                                    Zero, relu, sq, maxx, lower)
    from concourse.dve_spec import _has_src1
    from concourse.dve_uop import DveOpSpec
    from concourse.dve_ops import DveOp, OPS, CUSTOM_DVE_SPECS, _SUB_OPCODE_FOR_NAME

    def _make(name, spec):
        if name in _SUB_OPCODE_FOR_NAME:
            for op in OPS:
                if op.name == name:
                    return op
        shas = {}
        for ver in ("v3", "v4"):
            try:
                s = DveOpSpec(name=name, opcode=0, uops=lower(spec, ver=ver),
                              rd1_en=_has_src1(spec))
                shas[ver] = s.sha(ver)
            except Exception:
                pass
        op = DveOp(name, spec, subdim=False, uops_sha=shas)
        OPS.append(op)
        CUSTOM_DVE_SPECS[name] = spec
        _SUB_OPCODE_FOR_NAME[name] = max(_SUB_OPCODE_FOR_NAME.values()) + 1
        assert _SUB_OPCODE_FOR_NAME[name] < 0x20
        return op

    from concourse.dve_spec import C3, minn, _spill_c3_to_src1

    # sp tail: out = (relu(x+b) + u) + u^2*(q1 + u*q2),  u = exp(-|x+b|) from ACT
    def _sp_ref(in0, in1, s0, s1, imm2):
        x = in0.astype(np.float32) + s0
        u = in1.astype(np.float32)
        return (np.maximum(x, 0.0) + u) + (u * u) * (s1 + u * imm2)

    sp_op = _make("SOFTPLUS_TAIL", Spec(
        body=(relu(Src0 + C0) + Src1) + sq(Src1) * (C1 + Src1 * C2L),
        reference=_sp_ref))

    # a = min(|x+b|, clamp)
    def _absc_ref(in0, in1, s0, s1, imm2):
        t = in0.astype(np.float32) + s0
        return np.minimum(np.abs(t), s1)

    t_ = Src0 + C0
    abs_op = _make("ABS_CLAMP", Spec(
        body=minn(maxx(t_, Zero - t_), C1), reference=_absc_ref))

    # v = 1 + a*(c1 + a*(c2 + a*(c3 + a*c4)))  with c4 spilled to in1 [P,1]
    def _p4_ref(in0, in1, s0, s1, imm2):
        a = in0.astype(np.float32)
        c4 = in1.astype(np.float32)
        return 1.0 + a * (s0 + a * (s1 + a * (imm2 + a * c4)))

    p4_body = _spill_c3_to_src1(
        One + Src0 * (C0 + Src0 * (C1 + Src0 * (C2L + Src0 * C3))))
    p4_op = _make("EXP8_P4", Spec(body=p4_body, reference=_p4_ref))
    return sp_op, abs_op, p4_op


def _build(n_steps):
    import concourse.tile as tile
    import concourse.mybir as mybir
    from concourse import bacc

    f32 = mybir.dt.float32
    bf = mybir.dt.bfloat16
    AF = mybir.ActivationFunctionType
    AX = mybir.AxisListType
    ALU = mybir.AluOpType

    SP_OP, ABS_OP, P4_OP = _register_custom_ops()

    nc = bacc.Bacc("TRN2", target_bir_lowering=False, debug=False)

    w0t = nc.declare_dram_parameter("w0t", [H, WID], bf, isOutput=False)
    w1t = nc.declare_dram_parameter("w1t", [WID, WID], bf, isOutput=False)
    w2t = nc.declare_dram_parameter("w2t", [WID, H * IN], bf, isOutput=False)
    b2m = nc.declare_dram_parameter("b2m", [IN // 2, 2 * H], bf, isOutput=False)
    delta = nc.declare_dram_parameter("delta", [IN // 2, BL * IN // 2], bf, isOutput=False)
    p43 = nc.declare_dram_parameter("p43", [128, 1], f32, isOutput=False)
    b0c = nc.declare_dram_parameter("b0c", [WID, 1], f32, isOutput=False)
    b1c = nc.declare_dram_parameter("b1c", [WID, 1], f32, isOutput=False)
    dcoef = nc.declare_dram_parameter("dcoef", [NST, BL * IN], f32, isOutput=False)
    coef = nc.declare_dram_parameter("coef", [128, 6 * 7 * BL], f32, isOutput=False)
    cdiag = nc.declare_dram_parameter("cdiag", [128, 6], f32, isOutput=False)
    y0t = nc.declare_dram_parameter("y0t", [H, BL], f32, isOutput=False)
    yout = nc.declare_dram_parameter("yout", [H, BL], f32, isOutput=True)

    with tile.TileContext(nc) as tc, ExitStack() as ctx:
        const = ctx.enter_context(tc.tile_pool(name="const", bufs=1))
        sb = ctx.enter_context(tc.tile_pool(name="sb", bufs=3))
        hb = ctx.enter_context(tc.tile_pool(name="hb", bufs=3))
        drp = ctx.enter_context(tc.tile_pool(name="drp", bufs=4))
        mats = ctx.enter_context(tc.tile_pool(name="mats", bufs=3))
        pss = ctx.enter_context(tc.tile_pool(name="pss", bufs=2, space="PSUM"))
        ps3p = ctx.enter_context(tc.tile_pool(name="ps3p", bufs=2, space="PSUM"))

        w0t_t = const.tile([H, WID], bf)
        w1t_t = const.tile([WID, WID], bf)
        w2t_t = const.tile([WID, H * IN], bf)
        b2m_t = const.tile([IN // 2, 2 * H], bf)
        delta_t = const.tile([IN // 2, BL * IN // 2], bf)
        p43_t = const.tile([128, 1], f32)
        b0c_t = const.tile([WID, 1], f32)
        b1c_t = const.tile([WID, 1], f32)
        coef_t = const.tile([128, 6 * 7 * BL], f32)
        cdiag_t = const.tile([128, 6], f32)
        y0t_t = const.tile([H, BL], f32)
        for t_, d_ in ((w0t_t, w0t), (w1t_t, w1t), (w2t_t, w2t), (b2m_t, b2m),
                       (delta_t, delta), (p43_t, p43), (b0c_t, b0c), (b1c_t, b1c),
                       (coef_t, coef), (cdiag_t, cdiag), (y0t_t, y0t)):
            nc.sync.dma_start(t_[:], d_[:, :])
        coef_v = coef_t[:].rearrange("p (c g b) -> p c g b", c=6, g=7)

        yk = const.tile([128, 7 * BL], f32)     # group 0: y (f32 state), 1..6: k1..k6
        ykv = yk[:].rearrange("p (g b) -> p g b", g=7)
        ybf = const.tile([128, BL], bf)         # bf16 copy of current y
        nc.gpsimd.tensor_copy(ykv[:, 0, :], y0t_t[:])
        nc.gpsimd.tensor_copy(ybf[:], y0t_t[:])

        HIN = IN // 2      # 16 chunks per half
        HCOL = BL * HIN    # 128 columns per half

        def softplus(ps, bias_ap, tag):
            a = sb.tile([128, BL], f32, tag="a" + tag)
            nc.vector._custom_dve(ABS_OP, out=a[:], in0=ps[:],
                                  s0=bias_ap, s1=A_CLAMP, imm2=0.0)
            u = sb.tile([128, BL], f32, tag="u" + tag)
            nc.scalar.activation(u[:], a[:], AF.Exp, scale=-1.0)
            h = hb.tile([128, BL], bf, tag="h" + tag)
            nc.vector._custom_dve(SP_OP, out=h[:], in0=ps[:], in1=u[:],
                                  s0=bias_ap, s1=Q1, imm2=Q2)
            return h

        yin = None
        for n in range(n_steps):
            for j in range(1, 7):
                s = n * 6 + (j - 1)
                rhs = ybf if j == 1 else yin

                ps1 = pss.tile([128, BL], f32, tag="ps1")
                nc.tensor.matmul(ps1[:], w0t_t[:], rhs[:], start=True, stop=True)

                # delta-inits early: PE can run them while DVE does softplus
                p3a = ps3p.tile([128, HCOL], f32, tag="p3a")
                p3b = ps3p.tile([128, HCOL], f32, tag="p3b")
                nc.tensor.matmul(p3a[:], b2m_t[:, 0:H], delta_t[:],
                                 start=True, stop=False)
                nc.tensor.matmul(p3b[:], b2m_t[:, H:2 * H], delta_t[:],
                                 start=True, stop=False)

                h1 = softplus(ps1, b0c_t[:, 0:1], "1")

                ps2 = pss.tile([128, BL], f32, tag="ps2")
                nc.tensor.matmul(ps2[:], w1t_t[:], h1[:], start=True, stop=True)
                h2 = softplus(ps2, b1c_t[:, 0:1], "2")

                dr = drp.tile([128, BL * IN], f32, tag="dr")
                nc.sync.dma_start(dr[:], dcoef[s:s + 1, :].broadcast_to([128, BL * IN]))

                for i in range(HIN):
                    nc.tensor.matmul(p3a[:, i * BL:(i + 1) * BL],
                                     w2t_t[:, i * H:(i + 1) * H],
                                     h2[:], start=False, stop=(i == HIN - 1))
                for i in range(HIN, IN):
                    nc.tensor.matmul(p3b[:, (i - HIN) * BL:(i - HIN + 1) * BL],
                                     w2t_t[:, i * H:(i + 1) * H],
                                     h2[:], start=False, stop=(i == IN - 1))

                # tanh halves into one mat tile (half A overlaps half-B chunks)
                mat = mats.tile([128, BL * IN], f32, tag="mat")
                nc.scalar.activation(mat[:, 0:HCOL], p3a[:], AF.Tanh)
                nc.scalar.activation(mat[:, HCOL:], p3b[:], AF.Tanh)

                # partial RK combination over groups 0..j-1 (emitted here so the
                # scheduler slots it into the tanh window, not the front)
                if j >= 2:
                    yt = sb.tile([128, 7 * BL], f32, tag="yt")
                    ytv = yt[:].rearrange("p (g b) -> p g b", g=7)
                    nc.vector.tensor_mul(ytv[:, 0:j, :], ykv[:, 0:j, :],
                                         coef_v[:, j - 1, 0:j, :])
                    partial = sb.tile([128, BL], f32, tag="partial")
                    nc.vector.tensor_reduce(
                        partial[:],
                        yt[:].rearrange("p (g b) -> p b g", g=7)[:, :, 0:j],
                        axis=AX.X, op=ALU.add)
                    part_ap = partial[:]
                else:
                    part_ap = ykv[:, 0, :]

                tmp = mats.tile([128, BL * IN], f32, tag="tmp")
                nc.vector.tensor_mul(tmp[:], mat[:], dr[:])
                nc.vector.tensor_reduce(
                    ykv[:, j, :], tmp[:].rearrange("p (i b) -> p b i", b=BL),
                    axis=AX.X, op=ALU.add)

                if j < 6:
                    yin = hb.tile([128, BL], bf, tag="yin")
                    nc.vector.affine_then_add(yin[:], ykv[:, j, :], part_ap,
                                              scale=cdiag_t[:, j - 1:j], bias=0.0)
                else:
                    ynew = sb.tile([128, BL], f32, tag="ynew")
                    nc.vector.affine_then_add(ynew[:], ykv[:, j, :], part_ap,
                                              scale=cdiag_t[:, 5:6], bias=0.0)
                    nc.gpsimd.tensor_copy(ykv[:, 0, :], ynew[:])
                    nc.gpsimd.tensor_copy(ybf[:], ynew[:])

        nc.sync.dma_start(yout[:, :], yk[:, 0:BL])
    nc.compile()
    return nc


def _f32(x):
    return np.float32(x)


def _host_precompute(inputs):
    ts = np.asarray(inputs["ts"], np.float32)
    coeff_d = np.asarray(inputs["coeff_d"], np.float32)
    coeff_c = np.asarray(inputs["coeff_c"], np.float32)
    coeff_b = np.asarray(inputs["coeff_b"], np.float32)
    coeff_a = np.asarray(inputs["coeff_a"], np.float32)
    W0 = np.asarray(inputs["W0"], np.float32)
    W1 = np.asarray(inputs["W1"], np.float32)
    W2 = np.asarray(inputs["W2"], np.float32)
    b0 = np.asarray(inputs["b0"], np.float32)
    b1 = np.asarray(inputs["b1"], np.float32)
    b2 = np.asarray(inputs["b2"], np.float32)

    dt = _f32((ts[-1] - ts[0]) / _f32(N_STEPS))

    # dxdt at all stage times, f32 mirroring the jax reference arithmetic
    d_all = np.empty((NST, B, IN), np.float32)
    for n in range(N_STEPS):
        t0 = _f32(ts[0] + dt * _f32(n))
        for j in range(6):
            tt = _f32(t0 + _f32(C_OFFS[j]) * dt) if j > 0 else t0
            idx = int(np.clip(np.searchsorted(ts, tt, side="right") - 1, 0, T - 2))
            frac = _f32(tt - ts[idx])
            d_all[n * 6 + j] = (coeff_b[:, idx]
                                + frac * (_f32(2.0) * coeff_c[:, idx]
                                          + _f32(3.0) * frac * coeff_d[:, idx]))

    # initial MLP on host (f32, exact as reference)
    x0 = coeff_a[:, 0]
    h = np.maximum(x0 @ np.asarray(inputs["A0"], np.float32).T
                   + np.asarray(inputs["a0"], np.float32), 0)
    h = np.maximum(h @ np.asarray(inputs["A1"], np.float32).T
                   + np.asarray(inputs["a1"], np.float32), 0)
    y0 = (h @ np.asarray(inputs["A2"], np.float32).T
          + np.asarray(inputs["a2"], np.float32)).astype(np.float32)  # [B, H]

    # weights in device layouts
    w0t_np = np.ascontiguousarray(W0.T).astype(bf16)
    w1t_np = np.ascontiguousarray(W1.T).astype(bf16)
    W2r = W2.reshape(H, IN, WID)
    w2t_np = np.ascontiguousarray(W2r.transpose(2, 1, 0).reshape(WID, IN * H)).astype(bf16)
    b2m_full = b2.reshape(H, IN).T  # [IN, H]
    b2m_np = np.ascontiguousarray(
        np.concatenate([b2m_full[:IN // 2], b2m_full[IN // 2:]], axis=1)).astype(bf16)
    delta_np = np.repeat(np.eye(IN // 2, dtype=np.float32), BL, axis=1).astype(bf16)
    p43_np = np.full((128, 1), P4[3], np.float32)
    b0c_np = b0.reshape(WID, 1).copy()
    b1c_np = b1.reshape(WID, 1).copy()

    # RK combo coefficient tiles
    coef_np = np.zeros((128, 6, 7, BL), np.float32)
    for cj in range(6):
        coef_np[:, cj, 0, :] = 1.0
        row = A_ROWS[cj] if cj < 5 else B_ROW
        for l, a in enumerate(row):
            coef_np[:, cj, l + 1, :] = dt * _f32(a)
    coef_np = coef_np.reshape(128, 6 * 7 * BL)

    cdiag_np = np.zeros((128, 6), np.float32)
    for cj in range(6):
        row = A_ROWS[cj] if cj < 5 else B_ROW
        cdiag_np[:, cj] = dt * _f32(row[-1])

    per_core = []
    for c in range(N_CORES):
        bs = slice(c * BL, (c + 1) * BL)
        dcoef_np = np.ascontiguousarray(
            d_all[:, bs, :].transpose(0, 2, 1).reshape(NST, IN * BL)).astype(np.float32)
        y0t_np = np.ascontiguousarray(y0[bs].T)  # [H, BL]
        per_core.append(dict(
            w0t=w0t_np, w1t=w1t_np, w2t=w2t_np, b2m=b2m_np, delta=delta_np,
            p43=p43_np, b0c=b0c_np, b1c=b1c_np, dcoef=dcoef_np, coef=coef_np,
            cdiag=cdiag_np, y0t=y0t_np))
    return per_core, y0


def kernel(**inputs):
    from concourse.bass_utils import run_bass_kernel_spmd

    if "nc" not in _CACHE:
        _CACHE["nc"] = _build(N_STEPS)
    nc = _CACHE["nc"]

    in_maps, _ = _host_precompute(inputs)
    res = run_bass_kernel_spmd(nc, in_maps, core_ids=list(range(N_CORES)))
    _CACHE["last_result"] = res

    y = np.empty((B, H), np.float32)
    for c in range(N_CORES):
        y[c * BL:(c + 1) * BL] = res.results[c]["yout"].T

    Wl = np.asarray(inputs["Wl"], np.float32)
    bl = np.asarray(inputs["bl"], np.float32)
    logits = y @ Wl.T + bl
    out = (1.0 / (1.0 + np.exp(-logits)))[:, 0]
    return out.astype(np.float32)



# revision 6
# speedup vs baseline: 1.0560x; 1.0560x over previous
"""Neural CDE (Tsit5 scan over cubic-interp control) on 8 Trainium2 cores.

Strategy: pure data parallelism over batch (64 -> 8 per core), fp16 on-chip
arithmetic (4x less rounding noise than bf16 at identical engine speed).

Key structural points vs the v1 kernel:
- The Tsit5 stage combinations y_j = y + dt*sum(a_jl k_l) are folded into the
  PE as psum accumulation against pre-scaled weight copies Wa[j,l] = a_jl*W0:
  psum1_j = W0 ybf + sum_l Wa[j,l] @ (dt k_l partials). This removes the DVE
  partial-sum chain and the yin materialization from the critical path; all
  but the freshest-partial matmuls are emitted a stage early and execute in
  the shadow of the W2 chunk stream.
- The step update y_{n+1} = y + sum B_l dt k_l likewise becomes the stage-1
  psum accumulation of the next step via Wb[l] = B_l*W0 copies.
- b2 bias enters psum3 as a single rank-32 matmul (b2iT indicator trick).
- tanh/mul/reduce of the control einsum run per half (16 chunks) so they
  overlap the second half's chunk stream; the einsum multiply writes tmp in
  b-major order so the reduce is contiguous.
- softplus: DVE |x+b| (clamped at 88) -> ACT exp (psum-resident input) ->
  fused DVE tail (relu(x+b)+u)+u^2(Q1+u*Q2); tmp/mat/u stay f32, so the only
  16-bit roundings are the weights, h1/h2, dr, and the k partials.
"""

import numpy as np
import ml_dtypes
from contextlib import ExitStack

f16 = np.float16

# ---- problem constants (hardcoded per spec) ----
B, T, IN, H, WID, OUT = 64, 64, 32, 128, 128, 1
SUBSTEPS = 2
N_STEPS = (T - 1) * SUBSTEPS  # 126
N_CORES = 8
BL = B // N_CORES  # 8 batch per core
NST = N_STEPS * 6  # 756 stage evals

# Tsit5 tableau
C2, C3, C4, C5 = 0.161, 0.327, 0.9, 0.9800255409045097
A_ROWS = [
    [0.161],
    [-0.008480655492356989, 0.335480655492357],
    [2.8971530571054935, -6.359448489975075, 4.3622954328695815],
    [5.325864828439257, -11.748883564062828, 7.4955393428898365, -0.09249506636175525],
    [5.86145544294642, -12.92096931784711, 8.159367898576159, -0.071584973281401,
     -0.028269050394068383],
]
B_ROW = [0.09646076681806523, 0.01, 0.4798896504144996, 1.379008574103742,
         -3.290069515436081, 2.324710524099774]
C_OFFS = [0.0, C2, C3, C4, C5, 1.0]

# log1p(u) ~= u * (1 + u*(Q1 + u*Q2)) on (0, 1]  (max abs err ~1.3e-3)
Q1 = -0.44593992199872445
Q2 = 0.14039984369167596
A_CLAMP = 88.0  # keeps exp(-a) in the spline's domain; no accuracy effect

# family slot layout in wfam: Wa[j][l] = a_{j,l} * W0^T for j=2..6, l=1..j-1
# (15 slots), then Wb[l] = B_l * W0^T for l=1..6 (6 slots).
def _fam_slot(j, l):
    # j = target stage (2..6), l = k index (1..j-1)
    base = sum(range(1, j - 1))  # 0,1,3,6,10 for j=2..6
    return base + (l - 1)


def _famb_slot(l):
    return 15 + (l - 1)


N_FAM = 21

_CACHE = {}


def _register_custom_ops():
    from concourse.dve_spec import (Spec, Src0, Src1, C0, C1, C2 as C2L, Zero,
                                    relu, sq, maxx, minn)
    from concourse.dve_spec import _has_src1
    from concourse.dve_uop import DveOpSpec
    from concourse.dve_ops import DveOp, OPS, CUSTOM_DVE_SPECS, _SUB_OPCODE_FOR_NAME

    def _make(name, spec):
        if name in _SUB_OPCODE_FOR_NAME:
            for op in OPS:
                if op.name == name:
                    return op
        shas = {}
        for ver in ("v3", "v4"):
            try:
                s = DveOpSpec(name=name, opcode=0, uops=lower_spec(spec, ver=ver),
                              rd1_en=_has_src1(spec))
                shas[ver] = s.sha(ver)
            except Exception:
                pass
        op = DveOp(name, spec, subdim=False, uops_sha=shas)
        OPS.append(op)
        CUSTOM_DVE_SPECS[name] = spec
        _SUB_OPCODE_FOR_NAME[name] = max(_SUB_OPCODE_FOR_NAME.values()) + 1
        assert _SUB_OPCODE_FOR_NAME[name] < 0x20
        return op

    from concourse.dve_spec import lower as lower_spec

    # sp tail: out = (relu(x+b) + u) + u^2*(q1 + u*q2),  u = exp(-|x+b|) from ACT
    def _sp_ref(in0, in1, s0, s1, imm2):
        x = in0.astype(np.float32) + s0
        u = in1.astype(np.float32)
        return (np.maximum(x, 0.0) + u) + (u * u) * (s1 + u * imm2)

    sp_op = _make("SOFTPLUS_TAIL", Spec(
        body=(relu(Src0 + C0) + Src1) + sq(Src1) * (C1 + Src1 * C2L),
        reference=_sp_ref))

    # a = min(|x+b|, clamp)
    def _absc_ref(in0, in1, s0, s1, imm2):
        t = in0.astype(np.float32) + s0
        return np.minimum(np.abs(t), s1)

    t_ = Src0 + C0
    abs_op = _make("ABS_CLAMP", Spec(
        body=minn(maxx(t_, Zero - t_), C1), reference=_absc_ref))

    return sp_op, abs_op


def _build(n_steps):
    import concourse.tile as tile
    import concourse.mybir as mybir
    from concourse import bacc

    f32 = mybir.dt.float32
    fp16 = mybir.dt.float16
    AF = mybir.ActivationFunctionType
    AX = mybir.AxisListType
    ALU = mybir.AluOpType

    SP_OP, ABS_OP = _register_custom_ops()

    nc = bacc.Bacc("TRN2", target_bir_lowering=False, debug=False)

    w0t = nc.declare_dram_parameter("w0t", [H, WID], fp16, isOutput=False)
    w1t = nc.declare_dram_parameter("w1t", [WID, WID], fp16, isOutput=False)
    w2t = nc.declare_dram_parameter("w2t", [WID, H * IN], fp16, isOutput=False)
    wfam = nc.declare_dram_parameter("wfam", [H, N_FAM * WID], fp16, isOutput=False)
    b2iT = nc.declare_dram_parameter("b2iT", [IN, H], fp16, isOutput=False)
    eind = nc.declare_dram_parameter("eind", [IN, IN * BL], fp16, isOutput=False)
    b0c = nc.declare_dram_parameter("b0c", [WID, 1], f32, isOutput=False)
    b1c = nc.declare_dram_parameter("b1c", [WID, 1], f32, isOutput=False)
    bcoef = nc.declare_dram_parameter("bcoef", [128, 12 * BL], f32, isOutput=False)
    dcoef = nc.declare_dram_parameter("dcoef", [NST, BL * IN], fp16, isOutput=False)
    y0t = nc.declare_dram_parameter("y0t", [H, BL], f32, isOutput=False)
    y0bf = nc.declare_dram_parameter("y0bf", [H, BL], fp16, isOutput=False)
    yout = nc.declare_dram_parameter("yout", [H, BL], f32, isOutput=True)

    with tile.TileContext(nc) as tc, ExitStack() as ctx:
        const = ctx.enter_context(tc.tile_pool(name="const", bufs=1))
        hb = ctx.enter_context(tc.tile_pool(name="hb", bufs=3))
        mats = ctx.enter_context(tc.tile_pool(name="mats", bufs=2))
        tmps = ctx.enter_context(tc.tile_pool(name="tmps", bufs=2))
        drp = ctx.enter_context(tc.tile_pool(name="drp", bufs=4))
        pp = ctx.enter_context(tc.tile_pool(name="pp", bufs=2))
        scr = ctx.enter_context(tc.tile_pool(name="scr", bufs=2))
        p1p = ctx.enter_context(tc.tile_pool(name="p1p", bufs=2, space="PSUM"))
        p2p = ctx.enter_context(tc.tile_pool(name="p2p", bufs=1, space="PSUM"))
        p3p = ctx.enter_context(tc.tile_pool(name="p3p", bufs=2, space="PSUM"))
        pap = ctx.enter_context(tc.tile_pool(name="pap", bufs=1, space="PSUM"))

        w0t_t = const.tile([H, WID], fp16)
        w1t_t = const.tile([WID, WID], fp16)
        w2t_t = const.tile([WID, H * IN], fp16)
        wfam_t = const.tile([H, N_FAM * WID], fp16)
        b2iT_t = const.tile([IN, H], fp16)
        eind_t = const.tile([IN, IN * BL], fp16)
        b0c_t = const.tile([WID, 1], f32)
        b1c_t = const.tile([WID, 1], f32)
        bcoef_t = const.tile([128, 12 * BL], f32)
        y0t_t = const.tile([H, BL], f32)
        y0bf_t = const.tile([H, BL], fp16)
        for t_, d_ in ((w0t_t, w0t), (w1t_t, w1t), (w2t_t, w2t), (wfam_t, wfam),
                       (b2iT_t, b2iT), (eind_t, eind), (b0c_t, b0c), (b1c_t, b1c),
                       (bcoef_t, bcoef), (y0t_t, y0t), (y0bf_t, y0bf)):
            nc.sync.dma_start(t_[:], d_[:, :])

        def fam_ap(slot):
            return wfam_t[:, slot * WID:(slot + 1) * WID]

        # persistent state
        y_st = const.tile([H, BL], f32)
        ybf = const.tile([H, BL], fp16)
        nc.vector.tensor_copy(y_st[:], y0t_t[:])
        nc.vector.tensor_copy(ybf[:], y0bf_t[:])

        HB2 = IN * BL // 2  # half of the psum3 columns (16 chunks)

        def softplus(ps, bias_ap, tag):
            a = pap.tile([128, BL], f32, tag="a" + tag)
            nc.vector._custom_dve(ABS_OP, out=a[:], in0=ps[:],
                                  s0=bias_ap, s1=A_CLAMP, imm2=0.0)
            u = hb.tile([128, BL], f32, tag="u" + tag)
            nc.scalar.activation(u[:], a[:], AF.Exp, scale=-1.0)
            h = hb.tile([128, BL], fp16, tag="h" + tag)
            nc.vector._custom_dve(SP_OP, out=h[:], in0=ps[:], in1=u[:],
                                  s0=bias_ap, s1=Q1, imm2=Q2)
            return h

        # bootstrap: psum1 for (n=0, j=1) holds W0 @ y0
        p1_cur = p1p.tile([128, BL], f32, tag="p1")
        nc.tensor.matmul(p1_cur[:], w0t_t[:], y0bf_t[:], start=True, stop=True)

        P_cur = pp.tile([128, 12 * BL], fp16, tag="P")  # dt*k partials, (l, half)

        for n in range(n_steps):
            for j in range(1, 7):
                s = n * 6 + (j - 1)
                last = (n == n_steps - 1) and (j == 6)

                dr = drp.tile([128, BL * IN], fp16, tag="dr")
                nc.sync.dma_start(
                    dr[:], dcoef[s:s + 1, :].broadcast_to([128, BL * IN]))

                # ---- softplus(psum1) -> h1, mm2, softplus -> h2 ----
                h1 = softplus(p1_cur, b0c_t[:, 0:1], "1")
                p2 = p2p.tile([128, BL], f32, tag="p2")
                nc.tensor.matmul(p2[:], w1t_t[:], h1[:], start=True, stop=True)
                h2 = softplus(p2, b1c_t[:, 0:1], "2")

                # ---- psum3: b2 bias fill + 32 weight-chunk matmuls ----
                p3 = p3p.tile([128, IN * BL], f32, tag="p3")
                nc.tensor.matmul(p3[:], b2iT_t[:], eind_t[:],
                                 start=True, stop=False)
                for i in range(IN):
                    nc.tensor.matmul(p3[:, i * BL:(i + 1) * BL],
                                     w2t_t[:, i * H:(i + 1) * H],
                                     h2[:], start=False, stop=(i == IN - 1))

                # ---- family pre-accumulation for the NEXT stage's psum1 ----
                # next stage: j+1 within the step, or stage 1 of the next step
                # (then the Wb row applies and the base ybf is the OLD one).
                if not last:
                    p1_next = p1p.tile([128, BL], f32, tag="p1")
                    nc.tensor.matmul(p1_next[:], w0t_t[:], ybf[:],
                                     start=True, stop=False)
                    if j < 6:
                        slots = [_fam_slot(j + 1, l) for l in range(1, j + 1)]
                    else:
                        slots = [_famb_slot(l) for l in range(1, 7)]
                    # all but the last k are already reduced -> emit now
                    for li, slot in enumerate(slots[:-1]):
                        l = li + 1
                        for hf in range(2):
                            pl = P_cur[:, ((l - 1) * 2 + hf) * BL:
                                       ((l - 1) * 2 + hf + 1) * BL]
                            nc.tensor.matmul(p1_next[:], fam_ap(slot), pl,
                                             start=False, stop=False)

                # ---- tanh / mul / reduce per half; partials -> P_cur ----
                mat = mats.tile([128, IN * BL], f32, tag="mat")
                tmp = tmps.tile([128, BL * IN], f32, tag="tmp")
                tmpv = tmp[:].rearrange("p (b i) -> p b i", b=BL)
                for hf in range(2):
                    cs = slice(hf * HB2, (hf + 1) * HB2)
                    nc.scalar.activation(mat[:, cs], p3[:, cs], AF.Tanh)
                    # write the product b-major so the reduce is contiguous
                    tmph = tmpv[:, :, hf * (IN // 2):(hf + 1) * (IN // 2)]
                    tmph_ib = tmp[:].rearrange("p (b i) -> p i b", b=BL)[
                        :, hf * (IN // 2):(hf + 1) * (IN // 2), :]
                    nc.vector.tensor_mul(
                        tmph_ib,
                        mat[:, cs].rearrange("p (i b) -> p i b", b=BL),
                        dr[:, cs].rearrange("p (i b) -> p i b", b=BL))
                    pl = P_cur[:, ((j - 1) * 2 + hf) * BL:
                               ((j - 1) * 2 + hf + 1) * BL]
                    with nc.allow_low_precision("fp16 dt*k partials"):
                        nc.vector.tensor_reduce(
                            pl, tmph, axis=AX.X, op=ALU.add)
                    if not last:
                        slot = slots[-1]
                        nc.tensor.matmul(p1_next[:], fam_ap(slot), pl,
                                         start=False, stop=(hf == 1))

                # ---- step end: y state update (off critical path) ----
                if j == 6:
                    sc2 = scr.tile([128, 12 * BL], f32, tag="sc")
                    nc.vector.tensor_mul(sc2[:], P_cur[:], bcoef_t[:])
                    ts_ = scr.tile([128, BL], f32, tag="ts")
                    nc.vector.tensor_reduce(
                        ts_[:], sc2[:].rearrange("p (lh b) -> p b lh", b=BL),
                        axis=AX.X, op=ALU.add)
                    nc.vector.tensor_add(y_st[:], y_st[:], ts_[:])
                    nc.vector.tensor_copy(ybf[:], y_st[:])
                    if not last:
                        P_cur = pp.tile([128, 12 * BL], fp16, tag="P")

                if not last:
                    p1_cur = p1_next

        nc.sync.dma_start(yout[:, :], y_st[:])
    nc.compile()
    return nc


def _f32(x):
    return np.float32(x)


def _host_precompute(inputs):
    ts = np.asarray(inputs["ts"], np.float32)
    coeff_d = np.asarray(inputs["coeff_d"], np.float32)
    coeff_c = np.asarray(inputs["coeff_c"], np.float32)
    coeff_b = np.asarray(inputs["coeff_b"], np.float32)
    coeff_a = np.asarray(inputs["coeff_a"], np.float32)
    W0 = np.asarray(inputs["W0"], np.float32)
    W1 = np.asarray(inputs["W1"], np.float32)
    W2 = np.asarray(inputs["W2"], np.float32)
    b0 = np.asarray(inputs["b0"], np.float32)
    b1 = np.asarray(inputs["b1"], np.float32)
    b2 = np.asarray(inputs["b2"], np.float32)

    dt = _f32((ts[-1] - ts[0]) / _f32(N_STEPS))

    # dxdt at all stage times, f32 mirroring the jax reference arithmetic,
    # PRE-SCALED by dt so the on-device partials are dt*k directly.
    d_all = np.empty((NST, B, IN), np.float32)
    for n in range(N_STEPS):
        t0 = _f32(ts[0] + dt * _f32(n))
        for j in range(6):
            tt = _f32(t0 + _f32(C_OFFS[j]) * dt) if j > 0 else t0
            idx = int(np.clip(np.searchsorted(ts, tt, side="right") - 1, 0, T - 2))
            frac = _f32(tt - ts[idx])
            d_all[n * 6 + j] = (coeff_b[:, idx]
                                + frac * (_f32(2.0) * coeff_c[:, idx]
                                          + _f32(3.0) * frac * coeff_d[:, idx]))
    d_all *= dt

    # initial MLP on host (f32, exact as reference)
    x0 = coeff_a[:, 0]
    h = np.maximum(x0 @ np.asarray(inputs["A0"], np.float32).T
                   + np.asarray(inputs["a0"], np.float32), 0)
    h = np.maximum(h @ np.asarray(inputs["A1"], np.float32).T
                   + np.asarray(inputs["a1"], np.float32), 0)
    y0 = (h @ np.asarray(inputs["A2"], np.float32).T
          + np.asarray(inputs["a2"], np.float32)).astype(np.float32)  # [B, H]

    # weights in device layouts (fp16)
    w0t_np = np.ascontiguousarray(W0.T).astype(f16)
    w1t_np = np.ascontiguousarray(W1.T).astype(f16)
    W2r = W2.reshape(H, IN, WID)
    w2t_np = np.ascontiguousarray(
        W2r.transpose(2, 1, 0).reshape(WID, IN * H)).astype(f16)

    wfam_np = np.empty((H, N_FAM * WID), np.float32)
    for j in range(2, 7):
        for l in range(1, j):
            a = _f32(A_ROWS[j - 2][l - 1])
            wfam_np[:, _fam_slot(j, l) * WID:(_fam_slot(j, l) + 1) * WID] = a * W0.T
    for l in range(1, 7):
        bq = _f32(B_ROW[l - 1])
        wfam_np[:, _famb_slot(l) * WID:(_famb_slot(l) + 1) * WID] = bq * W0.T
    wfam_np = wfam_np.astype(f16)

    b2iT_np = np.ascontiguousarray(b2.reshape(H, IN).T).astype(f16)  # [IN, H]
    eind_np = np.repeat(np.eye(IN, dtype=np.float32), BL, axis=1).astype(f16)

    b0c_np = b0.reshape(WID, 1).copy()
    b1c_np = b1.reshape(WID, 1).copy()

    bcoef_np = np.zeros((128, 12 * BL), np.float32)
    for l in range(6):
        for hf in range(2):
            bcoef_np[:, (l * 2 + hf) * BL:(l * 2 + hf + 1) * BL] = _f32(B_ROW[l])

    per_core = []
    for c in range(N_CORES):
        bs = slice(c * BL, (c + 1) * BL)
        dcoef_np = np.ascontiguousarray(
            d_all[:, bs, :].transpose(0, 2, 1).reshape(NST, IN * BL)).astype(f16)
        y0t_np = np.ascontiguousarray(y0[bs].T)  # [H, BL]
        per_core.append(dict(
            w0t=w0t_np, w1t=w1t_np, w2t=w2t_np, wfam=wfam_np, b2iT=b2iT_np,
            eind=eind_np, b0c=b0c_np, b1c=b1c_np, bcoef=bcoef_np,
            dcoef=dcoef_np, y0t=y0t_np, y0bf=y0t_np.astype(f16)))
    return per_core, y0


def kernel(**inputs):
    from concourse.bass_utils import run_bass_kernel_spmd

    if "nc" not in _CACHE:
        _CACHE["nc"] = _build(N_STEPS)
    nc = _CACHE["nc"]

    in_maps, _ = _host_precompute(inputs)
    res = run_bass_kernel_spmd(nc, in_maps, core_ids=list(range(N_CORES)))
    _CACHE["last_result"] = res

    y = np.empty((B, H), np.float32)
    for c in range(N_CORES):
        y[c * BL:(c + 1) * BL] = res.results[c]["yout"].T

    Wl = np.asarray(inputs["Wl"], np.float32)
    bl = np.asarray(inputs["bl"], np.float32)
    logits = y @ Wl.T + bl
    out = (1.0 / (1.0 + np.exp(-logits)))[:, 0]
    return out.astype(np.float32)
